# revision 43
# baseline (speedup 1.0000x reference)
"""Trainium2 Bass kernel for nn_Attention (B=1, C=64, 12x12x12 spatial, 32 heads, head_dim=2).

Sharding: 32 heads split across 8 cores (4 heads/core), host sums the
8 partial w_proj outputs (tensor-parallel unshard, bias/8 per core).

Core design (v2, ACT-bound at ~91us of exp):
- Query blocks of (512,512,512,192), key chunks 13x128 + 64.
- One exp ACTIVATE per (block, chunk) covering all 4 heads (F=2048 from
  4 PSUM banks) -> amortizes the ~290-cycle per-instruction ACT overhead.
- PSUM managed manually as one [128,4096] tile: two 4-bank S buffers
  ping-pong (even/odd chunk). U_chunk matmuls are carved into bank 3 of
  the buffer ACT just finished (h3's region -- the LAST S matmul of the
  next same-parity chunk to touch it, so the DVE drain hides), proj and
  qkv staging into bank 2. DVE accumulates U into SBUF (u_acc) so no
  PSUM bank persists across the chunk loop.
- Software pipelining: S(kc+1) is emitted BEFORE U(kc) -- PE's queue is
  strict in-order, so the baseline's order (U before next S) serialized
  exp(kc) -> U(kc) -> S(kc+1) -> exp(kc+1) and starved ACT.
- Tail key chunk (64 keys) packs head pairs on partitions (rows 0:64 /
  64:128) halving its exp free-size; U uses block-diagonal V' weights.
- Per-head qkv matmuls run as a dense PE burst at t0 (warms the PE HAM
  clock gate; cold 1.2GHz PE was half the baseline's loss) and continue
  as carved pieces at chunk boundaries.
- Dummy 8-elem exp at t0 pulls the ~2.7us ACT table load under the
  input DMA. Divide uses reciprocal_approx_fast; divide+proj of block b
  run under block b+1's chunk loop so only the 192-block drains at the
  end.
"""

import numpy as np
import ml_dtypes

import concourse.bass as bass
import concourse.bacc as bacc
import concourse.mybir as mybir
from concourse import tile
from concourse.bass_utils import run_bass_kernel_spmd

C = 64
N = 1728
NCORES = 8
HLOC = 4
SCALE = float(2.0 ** -0.5)

KCS = [(i * 128, 128) for i in range(13)] + [(1664, 64)]
NKC = len(KCS)
QB = [(0, 512), (512, 512), (1024, 512), (1536, 192)]

F32 = mybir.dt.float32
BF16 = mybir.dt.bfloat16
EXPF = mybir.ActivationFunctionType.Exp


def build_nc():
    nc = bacc.Bacc(None)

    x2 = nc.declare_dram_parameter("x2", [C, N], BF16, isOutput=False)
    wq = nc.declare_dram_parameter("wq", [C, 2 * HLOC], BF16, isOutput=False)
    wk = nc.declare_dram_parameter("wk", [C, 2 * HLOC], BF16, isOutput=False)
    wv = nc.declare_dram_parameter("wv", [C, 2 * HLOC], BF16, isOutput=False)
    wp = nc.declare_dram_parameter("wp", [2 * HLOC + 1, C], F32, isOutput=False)
    y = nc.declare_dram_parameter("y", [C, N], F32, isOutput=True)

    with tile.TileContext(nc) as tc:
        with (
            tc.tile_pool(name="const", bufs=1) as cpool,
            tc.tile_pool(name="epool", bufs=3) as epool,
            tc.tile_pool(name="ps", bufs=1, space=bass.MemorySpace.PSUM) as pspool,
        ):
            x_sb = cpool.tile([C, N], BF16, name="x_sb")
            wq_sb = cpool.tile([C, 2 * HLOC], BF16, name="wq_sb")
            wk_sb = cpool.tile([C, 2 * HLOC], BF16, name="wk_sb")
            wv_sb = cpool.tile([C, 2 * HLOC], BF16, name="wv_sb")
            wp_sb = cpool.tile([2 * HLOC + 1, C], F32, name="wp_sb")
            qT = cpool.tile([128, N], BF16, name="qT")
            kT = cpool.tile([128, N], BF16, name="kT")
            qst = cpool.tile([2 * HLOC, N], BF16, name="qst")
            kst = cpool.tile([2 * HLOC, N], BF16, name="kst")
            vp = cpool.tile([128, NKC * HLOC * 3], BF16, name="vp")
            u_acc = cpool.tile([128, N], F32, name="u_acc")
            zot = cpool.tile([16, 512], F32, name="zot")
            zotr = cpool.tile([16, 512], F32, name="zotr")
            ot = cpool.tile([16, N], F32, name="ot")
            junk = cpool.tile([C, 1024], BF16, name="junk")
            ySB = cpool.tile([C, N], F32, name="ySB")
            dum = cpool.tile([1, 16], F32, name="dum")
            PS = pspool.tile([128, 4096], F32, name="PS")

            vp_v = vp[:].rearrange("p (kc h d) -> p kc h d", h=HLOC, d=3)

            def hg(t, c0, c1, r0, r1, g=32):
                """Partitions {g*h + r0..r1}, cols c0..c1 -> [4, r, c] view."""
                return t[:, c0:c1].rearrange("(h g) f -> h g f", g=g)[:, r0:r1, :]

            # ---- t0: ACT table prefetch first (nothing on the ACT queue
            # before the dummy exp), input DMAs on sync (spread over HW
            # queues), weights on gpsimd SWDGE ----
            nc.vector.memset(dum[:], 1.0)
            nc.scalar.activation(dum[0:1, 8:16], dum[0:1, 0:8], EXPF)
            nc.sync.dma_start(out=x_sb[:, 0:576], in_=x2[:, 0:576])
            nc.sync.dma_start(out=x_sb[:, 576:1152], in_=x2[:, 576:1152])
            nc.sync.dma_start(out=x_sb[:, 1152:N], in_=x2[:, 1152:N])
            nc.gpsimd.dma_start(out=wq_sb[:], in_=wq[:])
            nc.gpsimd.dma_start(out=wk_sb[:], in_=wk[:])
            nc.gpsimd.dma_start(out=wv_sb[:], in_=wv[:])
            nc.gpsimd.dma_start(out=wp_sb[:], in_=wp[:])
            # warm the PE pipeline + HAM clock gate while x is in flight
            nc.vector.memset(junk[:, :], 0.5)
            for i in range(3):
                nc.tensor.matmul(
                    PS[0:2, 2048 + 512 * (i % 2) : 2560 + 512 * (i % 2)],
                    junk[:, 0:2], junk[:, 2:514],
                    start=True, stop=True,
                )

            # ---- helpers ----
            def qk_piece(w_sb, dst, o, w, creg):
                """q or k for all 4 heads over x cols [o, o+w) via psum carve."""
                for h in range(HLOC):
                    nc.tensor.matmul(
                        PS[32 * h : 32 * h + 2, creg : creg + w],
                        w_sb[:, 2 * h : 2 * h + 2],
                        x_sb[:, o : o + w],
                        start=True, stop=True,
                        tile_position=(0, 32 * h),
                    )
                # contiguous partitions (DVE can't stride the partition dim);
                # junk rows between head groups land in unused qT/kT rows
                nc.vector.tensor_copy(
                    dst[0:98, o : o + w], PS[0:98, creg : creg + w]
                )

            def packed_piece(w_sb, stage, dstT, o, w, creg):
                """q or k for all heads in ONE matmul (out partitions 0:8 =
                (h,d)), then DVE->SBUF stage and DMA scatter into the
                32h-strided layout. 1 PE instruction instead of 4."""
                nc.tensor.matmul(
                    PS[0 : 2 * HLOC, creg : creg + w],
                    w_sb[:, 0 : 2 * HLOC],
                    x_sb[:, o : o + w],
                    start=True, stop=True,
                )
                nc.vector.tensor_copy(
                    stage[:, o : o + w], PS[0 : 2 * HLOC, creg : creg + w]
                )
                for d, eng in ((0, nc.sync), (1, nc.gpsimd)):
                    eng.dma_start(
                        out=dstT[:, o : o + w].rearrange("(h g) t -> h g t", g=32)[
                            :, d : d + 1, :
                        ],
                        in_=stage[:, o : o + w].rearrange("(h d) t -> h d t", d=2)[
                            :, d : d + 1, :
                        ],
                    )

            def vprime(k0, k1, base):
                for kc in range(k0, k1):
                    ko, kn = KCS[kc]
                    o = base + 8 * (kc - k0)
                    nc.tensor.matmul(
                        PS[0:kn, o : o + 8],
                        x_sb[:, ko : ko + kn],
                        wv_sb[:, 0 : 2 * HLOC],
                        start=True, stop=True,
                    )
                vsrc = PS[:, base : base + 8 * (k1 - k0)].rearrange(
                    "p (kc h d) -> p kc h d", h=HLOC, d=2
                )
                nc.vector.tensor_copy(vp_v[:, k0:k1, :, 0:2], vsrc)

            def emit_S(b, kc):
                qo, qn = QB[b]
                ko, kn = KCS[kc]
                buf = 0 if kc % 2 == 0 else 2048
                for h in range(HLOC):
                    nc.tensor.matmul(
                        PS[0:kn, buf + 512 * h : buf + 512 * h + qn],
                        kT[32 * h : 32 * h + 2, ko : ko + kn],
                        qT[32 * h : 32 * h + 2, qo : qo + qn],
                        start=True, stop=True,
                        tile_position=(32 * h, 0),
                    )

            def emit_exp(b, kc):
                qo, qn = QB[b]
                ko, kn = KCS[kc]
                buf = 0 if kc % 2 == 0 else 2048
                et = epool.tile([128, 2048], BF16, tag="e", name="et")
                if qn == 512:
                    nc.scalar.activation(
                        et[0:kn, 0:2048], PS[0:kn, buf : buf + 2048],
                        EXPF, scale=SCALE,
                    )
                else:
                    src = PS[0:kn, buf : buf + 2048].rearrange(
                        "p (h q) -> p h q", h=4
                    )[:, :, 0:qn]
                    dst = et[0:kn, 0 : 4 * qn].rearrange("p (h q) -> p h q", h=4)
                    nc.scalar.activation(dst, src, EXPF, scale=SCALE)
                return et

            def emit_U_add(b, kc, et):
                qo, qn = QB[b]
                ko, kn = KCS[kc]
                buf = 0 if kc % 2 == 0 else 2048
                cv = buf + 1536  # carve: bank 3 of the freed buffer
                for h in range(HLOC):
                    nc.tensor.matmul(
                        PS[32 * h : 32 * h + 3, cv : cv + qn],
                        vp_v[0:kn, kc, h, :],
                        et[0:kn, qn * h : qn * h + qn],
                        start=True, stop=True,
                        tile_position=(0, 32 * h),
                    )
                uc = PS[0:99, cv : cv + qn]
                ua = u_acc[0:99, qo : qo + qn]
                if kc == 0:
                    nc.vector.tensor_copy(ua, uc)
                else:
                    nc.vector.tensor_add(ua, ua, uc)

            def divide_piece(bprev, i, buf):
                qo, qn = QB[bprev]

                if i == 0:
                    # one parallel DMA wave: scatter U rows {32h+d} -> ot rows
                    # {2h+d}, and Z rows {32h+2} -> zot rows {2h+d}
                    for d, eng in ((0, nc.sync), (1, nc.gpsimd)):
                        otv = ot[0 : 2 * HLOC, qo : qo + qn].rearrange(
                            "(h g) f -> h g f", g=2
                        )[:, d : d + 1, :]
                        eng.dma_start(out=otv, in_=hg(u_acc, qo, qo + qn, d, d + 1))
                        ztv = zot[0 : 2 * HLOC, 0:qn].rearrange(
                            "(h g) f -> h g f", g=2
                        )[:, d : d + 1, :]
                        eng.dma_start(out=ztv, in_=hg(u_acc, qo, qo + qn, 2, 3))
                elif i == 1:
                    nc.vector.reciprocal_approx_fast(
                        zotr[0 : 2 * HLOC, 0 : qn // 2], zot[0 : 2 * HLOC, 0 : qn // 2]
                    )
                elif i == 2:
                    nc.vector.reciprocal_approx_fast(
                        zotr[0 : 2 * HLOC, qn // 2 : qn],
                        zot[0 : 2 * HLOC, qn // 2 : qn],
                    )
                elif i in (3, 4):
                    c0 = qo + (qn // 2) * (i - 3)
                    c1 = qo + qn if i == 4 else qo + qn // 2
                    z0 = (qn // 2) * (i - 3)
                    nc.vector.tensor_mul(
                        ot[0 : 2 * HLOC, c0:c1],
                        ot[0 : 2 * HLOC, c0:c1],
                        zotr[0 : 2 * HLOC, z0 : z0 + (c1 - c0)],
                    )
                elif i == 5:
                    # transposed proj: one matmul, y^T layout [C, tokens]
                    nc.tensor.matmul(
                        PS[0:C, buf + 1024 : buf + 1024 + qn],
                        wp_sb[:],
                        ot[0 : 2 * HLOC + 1, qo : qo + qn],
                        start=True, stop=True,
                    )
                    nc.vector.tensor_copy(
                        ySB[:, qo : qo + qn], PS[0:C, buf + 1024 : buf + 1024 + qn]
                    )
                elif i == 6:
                    nc.sync.dma_start(
                        out=y[:, qo : qo + qn], in_=ySB[:, qo : qo + qn]
                    )

            def specials(b, kc):
                """PE work carved into the freed buffer, emitted BEFORE the
                S-prefetch so it runs at window start (no dependencies)."""
                if b == 0:
                    base = 2048 if kc % 2 == 0 else 0  # freed buffer
                    if kc == 0:
                        vprime(0, 4, 2048)
                        packed_piece(wk_sb, kst, kT, 512, 512, 3584)
                    elif kc in (1, 2, 3, 4, 5):
                        vprime(2 + 2 * kc, 4 + 2 * kc, base)
                        if kc == 2:
                            packed_piece(wk_sb, kst, kT, 1024, 512, base + 512)
                        elif kc == 4:
                            packed_piece(wk_sb, kst, kT, 1536, 192, base + 512)
                    elif kc in (7, 9, 11):
                        w = 192 if kc == 11 else 512
                        packed_piece(wq_sb, qst, qT, 512 * ((kc - 5) // 2), w,
                                     base + 512)
                elif b < 3 and 2 <= kc <= 5:
                    base = 2048 if kc % 2 == 0 else 0
                    # junk matmul: keeps the PE busy through the DVE-heavy
                    # divide windows so the HAM clock gate stays unthrottled
                    nc.tensor.matmul(
                        PS[0:2, base + 1024 : base + 1536],
                        junk[:, 0:2], junk[:, 2:514],
                        start=True, stop=True,
                    )

            # ---- prologue PE burst (staged in bufB regions) ----
            packed_piece(wq_sb, qst, qT, 0, 512, 2560)   # q block0
            packed_piece(wk_sb, kst, kT, 0, 512, 3072)   # k chunks 0-3
            emit_S(0, 0)
            # memsets queue on gpsimd AFTER the prologue scatter DMAs
            nc.gpsimd.memset(ot[:, :], 1.0)
            nc.gpsimd.memset(vp_v[:, :, :, 2:3], 1.0)

            # ---- main loop (S software-pipelined one chunk ahead) ----
            for b in range(4):
                for kc in range(NKC):
                    buf = 0 if kc % 2 == 0 else 2048
                    et = emit_exp(b, kc)
                    specials(b, kc)
                    if kc < 13:
                        emit_S(b, kc + 1)
                    elif b < 3:
                        emit_S(b + 1, 0)
                    emit_U_add(b, kc, et)
                    if b > 0 and 1 <= kc <= 7:
                        divide_piece(b - 1, kc - 1, buf)

            # ---- tail: divide + proj + store for the final 192-block ----
            for i in range(7):
                divide_piece(3, i, 0)

    return nc


_NC = None


def _get_nc():
    global _NC
    if _NC is None:
        _NC = build_nc()
        _NC.finalize()
    return _NC


def make_in_maps(x, w_qkv, w_proj, b_proj):
    x2 = np.ascontiguousarray(x.reshape(C, N)).astype(ml_dtypes.bfloat16)
    in_maps = []
    for c in range(NCORES):
        sl = slice(8 * c, 8 * c + 8)
        wq = np.ascontiguousarray(w_qkv[sl, :].T).astype(ml_dtypes.bfloat16)
        wk = np.ascontiguousarray(w_qkv[64 + 8 * c : 64 + 8 * c + 8, :].T).astype(
            ml_dtypes.bfloat16
        )
        wv = np.ascontiguousarray(w_qkv[128 + 8 * c : 128 + 8 * c + 8, :].T).astype(
            ml_dtypes.bfloat16
        )
        wp = np.concatenate(
            [w_proj[:, sl].T, (b_proj / NCORES)[None, :]], axis=0
        ).astype(np.float32)
        in_maps.append(
            {"x2": x2, "wq": wq, "wk": wk, "wv": wv, "wp": np.ascontiguousarray(wp)}
        )
    return in_maps


def run(x, w_qkv, w_proj, b_proj, trace=False, **kw):
    nc = _get_nc()
    in_maps = make_in_maps(x, w_qkv, w_proj, b_proj)
    res = run_bass_kernel_spmd(
        nc, in_maps, core_ids=list(range(NCORES)), trace=trace, **kw
    )
    y = np.zeros((C, N), np.float32)
    for r in res.results:
        y += r["y"]
    return np.ascontiguousarray(y.T).reshape(1, 12, 12, 12, C), res


def kernel(x, w_qkv, w_proj, b_proj):
    out, _ = run(
        np.asarray(x), np.asarray(w_qkv), np.asarray(w_proj), np.asarray(b_proj)
    )
    return out


# revision 52
# speedup vs baseline: 1.0359x; 1.0359x over previous
"""Trainium2 Bass kernel for nn_Attention (B=1, C=64, 12x12x12 spatial, 32 heads, head_dim=2).

Sharding: 32 heads split across 8 cores (4 heads/core), host sums the
8 partial w_proj outputs (tensor-parallel unshard, bias/8 per core).

Core design (v2, ACT-bound at ~91us of exp):
- Query blocks of (512,512,512,192), key chunks 13x128 + 64.
- One exp ACTIVATE per (block, chunk) covering all 4 heads (F=2048 from
  4 PSUM banks) -> amortizes the ~290-cycle per-instruction ACT overhead.
- PSUM managed manually as one [128,4096] tile: two 4-bank S buffers
  ping-pong (even/odd chunk). U_chunk matmuls are carved into bank 3 of
  the buffer ACT just finished (h3's region -- the LAST S matmul of the
  next same-parity chunk to touch it, so the DVE drain hides), proj and
  qkv staging into bank 2. DVE accumulates U into SBUF (u_acc) so no
  PSUM bank persists across the chunk loop.
- Software pipelining: S(kc+1) is emitted BEFORE U(kc) -- PE's queue is
  strict in-order, so the baseline's order (U before next S) serialized
  exp(kc) -> U(kc) -> S(kc+1) -> exp(kc+1) and starved ACT.
- Tail key chunk (64 keys) packs head pairs on partitions (rows 0:64 /
  64:128) halving its exp free-size; U uses block-diagonal V' weights.
- Per-head qkv matmuls run as a dense PE burst at t0 (warms the PE HAM
  clock gate; cold 1.2GHz PE was half the baseline's loss) and continue
  as carved pieces at chunk boundaries.
- Dummy 8-elem exp at t0 pulls the ~2.7us ACT table load under the
  input DMA. Divide uses reciprocal_approx_fast; divide+proj of block b
  run under block b+1's chunk loop so only the 192-block drains at the
  end.
"""

import numpy as np
import ml_dtypes

import concourse.bass as bass
import concourse.bacc as bacc
import concourse.mybir as mybir
from concourse import tile
from concourse.bass_utils import run_bass_kernel_spmd

C = 64
N = 1728
NCORES = 8
HLOC = 4
SCALE = float(2.0 ** -0.5)

KCS = [(i * 128, 128) for i in range(13)] + [(1664, 64)]
NKC = len(KCS)
QB = [(0, 512), (512, 512), (1024, 512), (1536, 192)]

F32 = mybir.dt.float32
BF16 = mybir.dt.bfloat16
EXPF = mybir.ActivationFunctionType.Exp


def build_nc():
    nc = bacc.Bacc(None)

    x2 = nc.declare_dram_parameter("x2", [C, N], BF16, isOutput=False)
    wq = nc.declare_dram_parameter("wq", [C, 2 * HLOC], BF16, isOutput=False)
    wk = nc.declare_dram_parameter("wk", [C, 2 * HLOC], BF16, isOutput=False)
    wv = nc.declare_dram_parameter("wv", [C, 2 * HLOC], BF16, isOutput=False)
    wp = nc.declare_dram_parameter("wp", [2 * HLOC + 1, C], BF16, isOutput=False)
    y = nc.declare_dram_parameter("y", [C, N], F32, isOutput=True)

    with tile.TileContext(nc) as tc:
        with (
            tc.tile_pool(name="const", bufs=1) as cpool,
            tc.tile_pool(name="epool", bufs=3) as epool,
            tc.tile_pool(name="ps", bufs=1, space=bass.MemorySpace.PSUM) as pspool,
        ):
            x_sb = cpool.tile([C, N], BF16, name="x_sb")
            wq_sb = cpool.tile([C, 2 * HLOC], BF16, name="wq_sb")
            wk_sb = cpool.tile([C, 2 * HLOC], BF16, name="wk_sb")
            wv_sb = cpool.tile([C, 2 * HLOC], BF16, name="wv_sb")
            wp_sb = cpool.tile([2 * HLOC + 1, C], BF16, name="wp_sb")
            qT = cpool.tile([128, N], BF16, name="qT")
            kT = cpool.tile([128, N], BF16, name="kT")
            qst = cpool.tile([2 * HLOC, N], BF16, name="qst")
            kst = cpool.tile([2 * HLOC, N], BF16, name="kst")
            vp = cpool.tile([128, NKC * HLOC * 3], BF16, name="vp")
            u_acc = cpool.tile([128, N], F32, name="u_acc")
            zot = cpool.tile([16, 512], F32, name="zot")
            zotr = cpool.tile([16, 512], F32, name="zotr")
            ot32 = cpool.tile([16, N], F32, name="ot32")
            ot = cpool.tile([16, N], BF16, name="ot")
            junk = cpool.tile([C, 1024], BF16, name="junk")
            ySB = cpool.tile([C, N], F32, name="ySB")
            dum = cpool.tile([1, 16], F32, name="dum")
            PS = pspool.tile([128, 4096], F32, name="PS")

            vp_v = vp[:].rearrange("p (kc h d) -> p kc h d", h=HLOC, d=3)

            def hg(t, c0, c1, r0, r1, g=32):
                """Partitions {g*h + r0..r1}, cols c0..c1 -> [4, r, c] view."""
                return t[:, c0:c1].rearrange("(h g) f -> h g f", g=g)[:, r0:r1, :]

            # ---- t0: ACT table prefetch first (nothing on the ACT queue
            # before the dummy exp), input DMAs on sync (spread over HW
            # queues), weights on gpsimd SWDGE ----
            nc.vector.memset(dum[:], 1.0)
            nc.scalar.activation(dum[0:1, 8:16], dum[0:1, 0:8], EXPF)
            nc.sync.dma_start(out=x_sb[:, 0:576], in_=x2[:, 0:576])
            nc.sync.dma_start(out=x_sb[:, 576:1152], in_=x2[:, 576:1152])
            nc.sync.dma_start(out=x_sb[:, 1152:N], in_=x2[:, 1152:N])
            nc.gpsimd.dma_start(out=wq_sb[:], in_=wq[:])
            nc.gpsimd.dma_start(out=wk_sb[:], in_=wk[:])
            nc.gpsimd.dma_start(out=wv_sb[:], in_=wv[:])
            nc.gpsimd.dma_start(out=wp_sb[:], in_=wp[:])
            # warm the PE pipeline + HAM clock gate while x is in flight
            nc.vector.memset(junk[:, :], 0.5)
            for i in range(3):
                nc.tensor.matmul(
                    PS[0:2, 2048 + 512 * (i % 2) : 2560 + 512 * (i % 2)],
                    junk[:, 0:2], junk[:, 2:514],
                    start=True, stop=True,
                )

            # ---- helpers ----
            def qk_piece(w_sb, dst, o, w, creg):
                """q or k for all 4 heads over x cols [o, o+w) via psum carve."""
                for h in range(HLOC):
                    nc.tensor.matmul(
                        PS[32 * h : 32 * h + 2, creg : creg + w],
                        w_sb[:, 2 * h : 2 * h + 2],
                        x_sb[:, o : o + w],
                        start=True, stop=True,
                        tile_position=(0, 32 * h),
                    )
                # contiguous partitions (DVE can't stride the partition dim);
                # junk rows between head groups land in unused qT/kT rows
                nc.vector.tensor_copy(
                    dst[0:98, o : o + w], PS[0:98, creg : creg + w]
                )

            def packed_piece(w_sb, stage, dstT, o, w, creg):
                """q or k for all heads in ONE matmul (out partitions 0:8 =
                (h,d)), then DVE->SBUF stage and DMA scatter into the
                32h-strided layout. 1 PE instruction instead of 4."""
                nc.tensor.matmul(
                    PS[0 : 2 * HLOC, creg : creg + w],
                    w_sb[:, 0 : 2 * HLOC],
                    x_sb[:, o : o + w],
                    start=True, stop=True,
                )
                nc.vector.tensor_copy(
                    stage[:, o : o + w], PS[0 : 2 * HLOC, creg : creg + w]
                )
                for d, eng in ((0, nc.sync), (1, nc.gpsimd)):
                    eng.dma_start(
                        out=dstT[:, o : o + w].rearrange("(h g) t -> h g t", g=32)[
                            :, d : d + 1, :
                        ],
                        in_=stage[:, o : o + w].rearrange("(h d) t -> h d t", d=2)[
                            :, d : d + 1, :
                        ],
                    )

            def vprime(k0, k1, base):
                for kc in range(k0, k1):
                    ko, kn = KCS[kc]
                    o = base + 8 * (kc - k0)
                    nc.tensor.matmul(
                        PS[0:kn, o : o + 8],
                        x_sb[:, ko : ko + kn],
                        wv_sb[:, 0 : 2 * HLOC],
                        start=True, stop=True,
                    )
                vsrc = PS[:, base : base + 8 * (k1 - k0)].rearrange(
                    "p (kc h d) -> p kc h d", h=HLOC, d=2
                )
                nc.vector.tensor_copy(vp_v[:, k0:k1, :, 0:2], vsrc)

            def emit_S(b, kc):
                qo, qn = QB[b]
                ko, kn = KCS[kc]
                buf = 0 if kc % 2 == 0 else 2048
                for h in range(HLOC):
                    nc.tensor.matmul(
                        PS[0:kn, buf + 512 * h : buf + 512 * h + qn],
                        kT[32 * h : 32 * h + 2, ko : ko + kn],
                        qT[32 * h : 32 * h + 2, qo : qo + qn],
                        start=True, stop=True,
                        tile_position=(32 * h, 0),
                    )

            def emit_exp(b, kc):
                qo, qn = QB[b]
                ko, kn = KCS[kc]
                buf = 0 if kc % 2 == 0 else 2048
                et = epool.tile([128, 2048], BF16, tag="e", name="et")
                if qn == 512:
                    nc.scalar.activation(
                        et[0:kn, 0:2048], PS[0:kn, buf : buf + 2048],
                        EXPF, scale=SCALE,
                    )
                else:
                    src = PS[0:kn, buf : buf + 2048].rearrange(
                        "p (h q) -> p h q", h=4
                    )[:, :, 0:qn]
                    dst = et[0:kn, 0 : 4 * qn].rearrange("p (h q) -> p h q", h=4)
                    nc.scalar.activation(dst, src, EXPF, scale=SCALE)
                return et

            def emit_U_add(b, kc, et):
                qo, qn = QB[b]
                ko, kn = KCS[kc]
                buf = 0 if kc % 2 == 0 else 2048
                cv = buf + 1536  # carve: bank 3 of the freed buffer
                for h in range(HLOC):
                    nc.tensor.matmul(
                        PS[32 * h : 32 * h + 3, cv : cv + qn],
                        vp_v[0:kn, kc, h, :],
                        et[0:kn, qn * h : qn * h + qn],
                        start=True, stop=True,
                        tile_position=(0, 32 * h),
                    )
                uc = PS[0:99, cv : cv + qn]
                ua = u_acc[0:99, qo : qo + qn]
                if kc == 0:
                    nc.vector.tensor_copy(ua, uc)
                else:
                    nc.vector.tensor_add(ua, ua, uc)

            def divide_piece(bprev, i, buf):
                qo, qn = QB[bprev]

                if i == 0:
                    # one parallel DMA wave: scatter U rows {32h+d} -> ot32
                    # rows {2h+d}, and Z rows {32h+2} -> zot rows {2h+d}
                    for d, eng in ((0, nc.sync), (1, nc.gpsimd)):
                        otv = ot32[0 : 2 * HLOC, qo : qo + qn].rearrange(
                            "(h g) f -> h g f", g=2
                        )[:, d : d + 1, :]
                        eng.dma_start(out=otv, in_=hg(u_acc, qo, qo + qn, d, d + 1))
                        ztv = zot[0 : 2 * HLOC, 0:qn].rearrange(
                            "(h g) f -> h g f", g=2
                        )[:, d : d + 1, :]
                        eng.dma_start(out=ztv, in_=hg(u_acc, qo, qo + qn, 2, 3))
                elif i == 1:
                    nc.vector.reciprocal_approx_fast(
                        zotr[0 : 2 * HLOC, 0 : qn // 2], zot[0 : 2 * HLOC, 0 : qn // 2]
                    )
                elif i == 2:
                    nc.vector.reciprocal_approx_fast(
                        zotr[0 : 2 * HLOC, qn // 2 : qn],
                        zot[0 : 2 * HLOC, qn // 2 : qn],
                    )
                elif i in (3, 4):
                    c0 = qo + (qn // 2) * (i - 3)
                    c1 = qo + qn if i == 4 else qo + qn // 2
                    z0 = (qn // 2) * (i - 3)
                    nc.vector.tensor_mul(
                        ot[0 : 2 * HLOC, c0:c1],
                        ot32[0 : 2 * HLOC, c0:c1],
                        zotr[0 : 2 * HLOC, z0 : z0 + (c1 - c0)],
                    )
                elif i == 5:
                    # transposed proj: one matmul, y^T layout [C, tokens]
                    nc.tensor.matmul(
                        PS[0:C, buf + 1024 : buf + 1024 + qn],
                        wp_sb[:],
                        ot[0 : 2 * HLOC + 1, qo : qo + qn],
                        start=True, stop=True,
                    )
                    nc.vector.tensor_copy(
                        ySB[:, qo : qo + qn], PS[0:C, buf + 1024 : buf + 1024 + qn]
                    )
                elif i == 6:
                    nc.sync.dma_start(
                        out=y[:, qo : qo + qn], in_=ySB[:, qo : qo + qn]
                    )

            def specials(b, kc):
                """PE work carved into the freed buffer, emitted BEFORE the
                S-prefetch so it runs at window start (no dependencies)."""
                if b == 0:
                    base = 2048 if kc % 2 == 0 else 0  # freed buffer
                    if kc == 0:
                        vprime(0, 4, 2048)
                        packed_piece(wk_sb, kst, kT, 640, 384, 3712)
                    elif 2 <= kc <= 11:  # V' one chunk per window
                        vprime(kc + 2, kc + 3, base)
                        if kc == 3:
                            packed_piece(wk_sb, kst, kT, 1024, 512, base + 512)
                        elif kc == 5:
                            packed_piece(wk_sb, kst, kT, 1536, 192, base + 512)
                        elif kc in (7, 9, 11):
                            w = 192 if kc == 11 else 512
                            packed_piece(wq_sb, qst, qT, 512 * ((kc - 5) // 2),
                                         w, base + 512)
                elif b < 3 and 2 <= kc <= 5:
                    base = 2048 if kc % 2 == 0 else 0
                    # junk matmul: keeps the PE busy through the DVE-heavy
                    # divide windows so the HAM clock gate stays unthrottled
                    nc.tensor.matmul(
                        PS[0:2, base + 1024 : base + 1536],
                        junk[:, 0:2], junk[:, 2:514],
                        start=True, stop=True,
                    )

            # ---- prologue PE burst (staged in bufB regions) ----
            qk_piece(wq_sb, qT, 0, 512, 2560)            # q block0
            qk_piece(wk_sb, kT, 0, 128, 3584)            # k chunk 0
            packed_piece(wk_sb, kst, kT, 128, 512, 3072)  # k chunks 1-4
            emit_S(0, 0)
            # memsets queue on gpsimd AFTER the prologue scatter DMAs
            nc.gpsimd.memset(ot[:, :], 1.0)
            nc.gpsimd.memset(vp_v[:, :, :, 2:3], 1.0)

            # ---- main loop (S software-pipelined one chunk ahead) ----
            for b in range(4):
                for kc in range(NKC):
                    buf = 0 if kc % 2 == 0 else 2048
                    et = emit_exp(b, kc)
                    specials(b, kc)
                    if kc < 13:
                        emit_S(b, kc + 1)
                    elif b < 3:
                        emit_S(b + 1, 0)
                    emit_U_add(b, kc, et)
                    if b > 0 and 1 <= kc <= 7:
                        divide_piece(b - 1, kc - 1, buf)

            # ---- tail: divide + proj + store for the final 192-block ----
            for i in range(7):
                divide_piece(3, i, 0)

    return nc


_NC = None


def _get_nc():
    global _NC
    if _NC is None:
        _NC = build_nc()
        _NC.finalize()
    return _NC


def make_in_maps(x, w_qkv, w_proj, b_proj):
    x2 = np.ascontiguousarray(x.reshape(C, N)).astype(ml_dtypes.bfloat16)
    in_maps = []
    for c in range(NCORES):
        sl = slice(8 * c, 8 * c + 8)
        wq = np.ascontiguousarray(w_qkv[sl, :].T).astype(ml_dtypes.bfloat16)
        wk = np.ascontiguousarray(w_qkv[64 + 8 * c : 64 + 8 * c + 8, :].T).astype(
            ml_dtypes.bfloat16
        )
        wv = np.ascontiguousarray(w_qkv[128 + 8 * c : 128 + 8 * c + 8, :].T).astype(
            ml_dtypes.bfloat16
        )
        wp = np.concatenate(
            [w_proj[:, sl].T, (b_proj / NCORES)[None, :]], axis=0
        ).astype(ml_dtypes.bfloat16)
        in_maps.append(
            {"x2": x2, "wq": wq, "wk": wk, "wv": wv, "wp": np.ascontiguousarray(wp)}
        )
    return in_maps


def run(x, w_qkv, w_proj, b_proj, trace=False, **kw):
    nc = _get_nc()
    in_maps = make_in_maps(x, w_qkv, w_proj, b_proj)
    res = run_bass_kernel_spmd(
        nc, in_maps, core_ids=list(range(NCORES)), trace=trace, **kw
    )
    y = np.zeros((C, N), np.float32)
    for r in res.results:
        y += r["y"]
    return np.ascontiguousarray(y.T).reshape(1, 12, 12, 12, C), res


def kernel(x, w_qkv, w_proj, b_proj):
    out, _ = run(
        np.asarray(x), np.asarray(w_qkv), np.asarray(w_proj), np.asarray(b_proj)
    )
    return out
